# revision 2
# baseline (speedup 1.0000x reference)
"""MHSA Trainium2 Bass kernel, head-parallel over 8 NeuronCores.

x [4, 2048, 1024], W_qkv [1024, 3072], W_proj [1024, 1024], b_proj [1024];
H=16 heads, hd=64. Core c owns heads {2c, 2c+1} (128 feature dims).

Per-core program (SPMD; cores differ only in input data):
  1. QKV projection, feature-major: qT/kT [128, 8192] accumulated over 8
     D-slices (lhsT = W-slice [128, 128], rhs = xT chunk [128, 256]).
     V is PE-transposed to token-major and augmented with a ones column
     (row 64 of the PV output then carries the softmax denominator).
  2. Attention per (batch, head): S^T tile [k 128, q 512] via K=64 matmul;
     exp on ACT (scale=1/8, no max subtraction -- scores are O(1));
     PV accumulates psum [65, 512] over 16 k-tiles; reciprocal of row 64;
     PE outer-product broadcasts it; DVE multiply writes outT (fp32r).
  3. Projection partial [8192, 1024] = outT.T-slices @ W_proj-slice.
Host sums the 8 per-core partials and adds b_proj.

float32r operands run the PE at full rate for free-dim >= 256.
"""
import sys
sys.path.insert(0, "/opt/trn_rl_repo")
import numpy as np
import concourse.bass as bass
import concourse.mybir as mybir
import concourse.tile as tile
from concourse import bacc
from concourse.masks import make_identity
from concourse.bass_utils import run_bass_kernel_spmd

F32 = mybir.dt.float32
F32R = mybir.dt.float32r
AF = mybir.ActivationFunctionType

B, N, D = 4, 2048, 1024
H, HD = 16, 64
NC_CORES = 8
FPC = 128                               # feature dims per core (2 heads)
TOK = B * N                             # 8192
SCALE = HD ** -0.5

_CACHED = {}


def _build(trace=False):
    nc = bacc.Bacc(None)
    xT = nc.declare_dram_parameter("xT", [D, TOK], F32, isOutput=False)
    wq = nc.declare_dram_parameter("wq", [D, FPC], F32, isOutput=False)
    wk = nc.declare_dram_parameter("wk", [D, FPC], F32, isOutput=False)
    wv = nc.declare_dram_parameter("wv", [D, FPC], F32, isOutput=False)
    wp = nc.declare_dram_parameter("wp", [FPC, D], F32, isOutput=False)
    out = nc.declare_dram_parameter("out", [TOK, D], F32, isOutput=True)

    NTT = TOK // 128                    # 64 token tiles
    NQ1 = 256                           # phase-1 token chunk
    NQ = 512                            # phase-2/3 free dim
    NKT = N // 128                      # 16 k tiles per batch

    with nc.allow_low_precision(reason="fp32r matmul inputs; accum fp32"), \
         tile.TileContext(nc) as tc:
        with tc.tile_pool(name="big", bufs=1) as big, \
             tc.tile_pool(name="stage", bufs=2) as stage, \
             tc.tile_pool(name="work", bufs=3) as work, \
             tc.tile_pool(name="ps", bufs=2, space="PSUM") as ps:

            qT = big.tile([128, TOK], F32R)
            kT = big.tile([128, TOK], F32R)
            vaug = big.tile([128, NTT, 2, 65], F32R)
            outT = big.tile([128, TOK], F32R)
            ident = big.tile([128, 128], F32)
            make_identity(nc, ident)
            ones_f = big.tile([128, 1], F32)
            nc.vector.memset(ones_f, 1.0)
            ones1 = big.tile([1, 64], F32R)
            nc.vector.tensor_copy(ones1, ones_f[0:1, 0:1].to_broadcast([1, 64]))
            # ones columns of v_aug (denominator trick)
            nc.vector.tensor_copy(
                vaug[:, :, :, 64:65],
                ones_f[:, 0:1].to_broadcast([128, NTT, 2, 1]))

            wq_r = big.tile([128, 8, FPC], F32R)
            wk_r = big.tile([128, 8, FPC], F32R)
            wv_r = big.tile([128, 8, FPC], F32R)
            wp_r = big.tile([128, D], F32R)
            nc.sync.dma_start(out=wq_r, in_=wq.rearrange(
                "(s p) f -> p s f", p=128).bitcast(F32R))
            nc.sync.dma_start(out=wk_r, in_=wk.rearrange(
                "(s p) f -> p s f", p=128).bitcast(F32R))
            nc.sync.dma_start(out=wv_r, in_=wv.rearrange(
                "(s p) f -> p s f", p=128).bitcast(F32R))
            nc.sync.dma_start(out=wp_r, in_=wp[:, :].bitcast(F32R))

            # --- phase 1: QKV projection (feature-major) + V transpose ---
            for chg in range(TOK // NQ1):
                lo = chg * NQ1
                xr = stage.tile([128, 8, NQ1], F32R, tag="xr")
                nc.sync.dma_start(
                    out=xr,
                    in_=xT[:, lo:lo + NQ1]
                        .rearrange("(s p) n -> p s n", p=128).bitcast(F32R))
                pq = ps.tile([128, NQ1], F32, tag="psA")
                pk = ps.tile([128, NQ1], F32, tag="psB")
                pv = ps.tile([128, NQ1], F32, tag="psC")
                for s in range(8):
                    nc.tensor.matmul(pq, wq_r[:, s, :], xr[:, s, :],
                                     start=(s == 0), stop=(s == 7))
                for s in range(8):
                    nc.tensor.matmul(pk, wk_r[:, s, :], xr[:, s, :],
                                     start=(s == 0), stop=(s == 7))
                for s in range(8):
                    nc.tensor.matmul(pv, wv_r[:, s, :], xr[:, s, :],
                                     start=(s == 0), stop=(s == 7))
                nc.vector.tensor_copy(qT[:, lo:lo + NQ1], pq)
                nc.vector.tensor_copy(kT[:, lo:lo + NQ1], pk)
                vt_f = stage.tile([128, NQ1], F32, tag="vtf")
                nc.vector.tensor_copy(vt_f, pv)
                for tt in range(NQ1 // 128):
                    tok_tile = chg * (NQ1 // 128) + tt
                    pvt = ps.tile([128, 128], F32, tag="psD")
                    nc.tensor.matmul(
                        pvt, vt_f[:, tt * 128:(tt + 1) * 128], ident,
                        is_transpose=True, start=True, stop=True)
                    nc.vector.tensor_copy(vaug[:, tok_tile, 0, 0:64],
                                          pvt[:, 0:64])
                    nc.vector.tensor_copy(vaug[:, tok_tile, 1, 0:64],
                                          pvt[:, 64:128])

            # --- phase 2: attention per (batch, head) ---
            for b in range(B):
                for h in range(2):
                    hp = h * 64
                    for qc in range(N // NQ):
                        q_lo = b * N + qc * NQ
                        po = ps.tile([65, NQ], F32, tag="psB")
                        for kt in range(NKT):
                            k_lo = b * N + kt * 128
                            pst = ps.tile([128, NQ], F32, tag="psA")
                            nc.tensor.matmul(
                                pst,
                                kT[hp:hp + 64, k_lo:k_lo + 128],
                                qT[hp:hp + 64, q_lo:q_lo + NQ],
                                start=True, stop=True)
                            et = work.tile([128, NQ], F32, tag="et")
                            nc.scalar.activation(et, pst, AF.Exp,
                                                 bias=0.0, scale=SCALE)
                            er = work.tile([128, NQ], F32R, tag="er")
                            nc.vector.tensor_copy(er, et)
                            ktile = (b * N) // 128 + kt
                            nc.tensor.matmul(
                                po, vaug[:, ktile, h, :], er,
                                start=(kt == 0), stop=(kt == NKT - 1))
                        rec = work.tile([1, NQ], F32R, tag="rec", bufs=2)
                        nc.vector.reciprocal(rec, po[64:65, :])
                        pb = ps.tile([64, NQ], F32, tag="psC")
                        nc.tensor.matmul(pb, ones1, rec, start=True, stop=True)
                        bc = work.tile([64, NQ], F32, tag="bc", bufs=2)
                        nc.vector.tensor_copy(bc, pb)
                        nc.vector.tensor_mul(
                            outT[hp:hp + 64, q_lo:q_lo + NQ],
                            po[0:64, :], bc)

            # --- phase 3: projection partial ---
            for tt in range(NTT):
                for oc in range(D // NQ):
                    pp = ps.tile([128, NQ], F32, tag="psA")
                    nc.tensor.matmul(
                        pp, outT[:, tt * 128:(tt + 1) * 128],
                        wp_r[:, oc * NQ:(oc + 1) * NQ],
                        start=True, stop=True)
                    ob = work.tile([128, NQ], F32, tag="ob", bufs=2)
                    nc.vector.tensor_copy(ob, pp)
                    nc.sync.dma_start(
                        out=out[tt * 128:(tt + 1) * 128,
                                oc * NQ:(oc + 1) * NQ],
                        in_=ob)
    nc.finalize()
    return nc


def _in_maps(x, W_qkv, W_proj):
    xTm = np.ascontiguousarray(x.reshape(TOK, D).T)
    maps = []
    for c in range(NC_CORES):
        h0 = 2 * c
        cols = np.arange(h0 * HD, (h0 + 2) * HD)
        maps.append({
            "xT": xTm,
            "wq": np.ascontiguousarray(W_qkv[:, cols]),
            "wk": np.ascontiguousarray(W_qkv[:, D + cols]),
            "wv": np.ascontiguousarray(W_qkv[:, 2 * D + cols]),
            "wp": np.ascontiguousarray(W_proj[h0 * HD:(h0 + 2) * HD, :]),
        })
    return maps


def kernel(x, W_qkv, W_proj, b_proj, _trace=False):
    x = np.asarray(x, dtype=np.float32)
    W_qkv = np.asarray(W_qkv, dtype=np.float32)
    W_proj = np.asarray(W_proj, dtype=np.float32)
    b_proj = np.asarray(b_proj, dtype=np.float32)

    if "nc" not in _CACHED:
        _CACHED["nc"] = _build()
    nc = _CACHED["nc"]

    res = run_bass_kernel_spmd(nc, _in_maps(x, W_qkv, W_proj),
                               list(range(NC_CORES)), trace=_trace)
    acc = np.zeros((TOK, D), dtype=np.float32)
    for c in range(NC_CORES):
        acc += res.results[c]["out"]
    acc += b_proj[None, :]
    if _trace:
        return acc.reshape(B, N, D), res
    return acc.reshape(B, N, D)
